# revision 1
# baseline (speedup 1.0000x reference)
"""Trainium2 Bass kernel for nn_Attention_7653631722097.

Reference computation (per batch b of 8):
    qkv = silu(w_qkv @ x_b + b_qkv)            # [768, 1024], x_b = x[b] as [256, HW=1024]
    per head n (8 heads, ch=32): q,k,v = qkv[96n:96n+32], [+32:64], [+64:96]
    sT = (k^T q) / sqrt(32)                    # [1024(t), 1024(s)]
    p = exp(sT); sums = p.sum(axis=t)          # softmax denominator (no max-sub: |sT| < 1)
    pv = v @ p                                 # [32, 1024] unnormalized
    hid[32n:32n+32] = pv / sums
    out_b = w_out @ hid + b_out + x_b

Distribution: data-parallel over batch -> 1 batch per NeuronCore, 8 cores,
no collectives. All matmuls run in float32r (full-rate fp32 PE mode).

Layout strategy (everything stays at partition base 0 or a matched 32-aligned
base, so no partition-shifting ops are needed):
  - host passes weights pre-transposed and head-grouped:
      wqT/wkT [256(c), 256(o)]: lhsT for the q/k projections (o head-grouped)
      wvT     [256(c), 256(o)]: rhs so v is produced TRANSPOSED: vT[t, o_v]
      woT     [32, 8, 256]: per-head lhsT slices for the output projection
  - sT = k^T q via lhsT=k[32, tblk] rhs=q[32, :]  (both base 32*(n%4))
  - PV lhsT = [vT_head | ones] ([128, 33]) -> psum rows 0-31 = pv, row 32 = sums
  - biases are added via K=1 matmuls (ones outer products); silu = sigmoid*x
"""
import sys

sys.path.insert(0, "/opt/trn_rl_repo")

import numpy as np

B, C, H, W = 8, 256, 32, 32
NH, CH = 8, 32
S = H * W  # 1024
SCALE = 1.0 / np.sqrt(np.float32(CH))

_CACHE = {}


def _emit_body(nc, tc, mybir, tiles):
    """One batch worth of compute. Called once (fast path) or per loop
    iteration (timing variant)."""
    F32 = mybir.dt.float32
    F32R = mybir.dt.float32r
    AF = mybir.ActivationFunctionType
    x_t, wq_t, wk_t, wv_t, wo_t, br_t, on_t, out_d = tiles
    qksb, vtsb, sgsb, etsb, pvsb, rbsb, osb = (
        tc._k_pools[k]
        for k in ("qksb", "vtsb", "sgsb", "etsb", "pvsb", "rbsb", "osb")
    )

    # ---- phase Q: q/k projections + silu, and vT + silu --------
    q_t = [qksb.tile([128, S], F32R, tag=f"q{i}", name=f"q_t{i}") for i in range(2)]
    k_t = [qksb.tile([128, S], F32R, tag=f"k{i}", name=f"k_t{i}") for i in range(2)]
    vt_t = []

    with (
        tc.tile_pool(name="qkps", bufs=3, space="PSUM") as qkps,
        tc.tile_pool(name="vtps", bufs=2, space="PSUM") as vtps,
    ):
        def emit_qk(part, w_t, dsts, g):
            if True:
                ps = qkps.tile([128, S], F32, name=f"qkp_{part}_{g}", tag="qkp")
                for c in range(2):
                    cs = slice(512 * c, 512 * c + 512)
                    for kc in range(2):
                        nc.tensor.matmul(
                            ps[:, cs],
                            w_t[kc][:, 128 * g : 128 * g + 128],
                            x_t[kc][:, cs],
                            start=(kc == 0),
                            stop=False,
                        )
                    nc.tensor.matmul(
                        ps[:, cs],
                        br_t[0:1, part, 128 * g : 128 * g + 128],
                        on_t[0:1, cs],
                        start=False,
                        stop=True,
                    )
                sg = sgsb.tile([128, S], F32, tag="sg", name=f"sg_{part}_{g}")
                for c in range(2):
                    cs = slice(512 * c, 512 * c + 512)
                    nc.scalar.activation(
                        out=sg[:, cs], in_=ps[:, cs], func=AF.Sigmoid
                    )
                    nc.vector.tensor_mul(dsts[g][:, cs], sg[:, cs], ps[:, cs])

        emit_qk(0, wq_t, q_t, 0)
        emit_qk(1, wk_t, k_t, 0)
        for j in range(8):
            vps = vtps.tile([128, 256], F32, name=f"vps_{j}", tag="vps")
            ts = slice(128 * j, 128 * j + 128)
            for kc in range(2):
                nc.tensor.matmul(
                    vps[:],
                    x_t[kc][:, ts],
                    wv_t[kc][:],
                    start=(kc == 0),
                    stop=False,
                )
            nc.tensor.matmul(
                vps[:],
                on_t[0:1, 0:128],
                br_t[0:1, 2, :],
                start=False,
                stop=True,
            )
            sgv = sgsb.tile([128, 256], F32, tag="sgv", name=f"sgv_{j}")
            nc.scalar.activation(out=sgv[:], in_=vps[:], func=AF.Sigmoid)
            vt_j = vtsb.tile([128, NH, CH + 1], F32R, tag="vt", name=f"vt_{j}")
            nc.vector.tensor_mul(
                vt_j[:, :, 0:CH],
                sgv.rearrange("p (n c) -> p n c", n=NH),
                vps.rearrange("p (n c) -> p n c", n=NH),
            )
            # ones column for the fused softmax-denominator row
            nc.vector.tensor_copy(
                vt_j[:, :, CH : CH + 1],
                on_t[:, 0:NH].rearrange("p (n o) -> p n o", o=1),
            )
            vt_t.append(vt_j)
        emit_qk(0, wq_t, q_t, 1)
        emit_qk(1, wk_t, k_t, 1)

    # ---- phase A: attention per head ---------------------------
    pvu = []
    with (
        tc.tile_pool(name="stps", bufs=2, space="PSUM") as stps,
        tc.tile_pool(name="pvps", bufs=2, space="PSUM") as pvps,
    ):
        pv_t = {}

        def emit_norm(n):
            pvu_n = pvsb.tile([CH + 1, S], F32R, tag="pvu", name=f"pvu_{n}")
            rb = rbsb.tile([CH, S], F32, tag="rb", name=f"rb_{n}")
            rs0 = rbsb.tile([1, S], F32, tag="rs0", name=f"rs0_{n}")
            for c in range(2):
                cs = slice(512 * c, 512 * c + 512)
                nc.vector.tensor_copy(pvu_n[:, cs], pv_t[n][:, cs])
                # 1/sums written to partition 0 (partition_broadcast on HW
                # only accepts a base-partition-0 source)
                with nc.allow_low_precision(reason="f32 recip"):
                    nc.vector.reciprocal(
                        out=rs0[0:1, cs], in_=pvu_n[CH : CH + 1, cs].bitcast(F32)
                    )
                # broadcast 1/sums across 32 partitions on the idle GPSIMD
                # engine, then normalize pv in place
                nc.gpsimd.partition_broadcast(rb[:, cs], rs0[0:1, cs])
                with nc.allow_low_precision(reason="f32r norm, 4-byte"):
                    nc.vector.tensor_mul(
                        pvu_n[0:CH, cs], pvu_n[0:CH, cs], rb[:, cs]
                    )
            pvu.append(pvu_n)

        def emit_pv(n, j, et):
            for c in range(2):
                cs = slice(512 * c, 512 * c + 512)
                nc.tensor.matmul(
                    pv_t[n][:, cs],
                    vt_t[j][:, n, :],
                    et[:, cs],
                    start=(j == 0),
                    stop=(j == 7),
                )

        prev = None  # (n, j, et) whose PV is not yet emitted
        for n in range(NH):
            g, m = divmod(n, 4)
            rs = slice(32 * m, 32 * m + 32)
            pv_t[n] = pvps.tile([CH + 1, S], F32, name=f"pv_{n}", tag="pv")
            for j in range(8):
                st = stps.tile([128, S], F32, name=f"st_{n}_{j}", tag="st")
                for c in range(2):
                    cs = slice(512 * c, 512 * c + 512)
                    nc.tensor.matmul(
                        st[:, cs],
                        k_t[g][rs, 128 * j : 128 * j + 128],
                        q_t[g][rs, cs],
                        start=True,
                        stop=True,
                        tile_position=(32 * m, 0),
                    )
                et = etsb.tile([128, S], F32R, tag="et", name=f"et_{n}_{j}")
                nc.scalar.activation(
                    out=et[:], in_=st[:], func=AF.Exp, scale=float(SCALE)
                )
                if prev is not None:
                    emit_pv(*prev)
                    if prev[1] == 7:
                        emit_norm(prev[0])
                prev = (n, j, et)
        emit_pv(*prev)
        emit_norm(prev[0])



    # ---- phase O: output projection + residual ------------------
    with tc.tile_pool(name="ocps", bufs=2, space="PSUM") as ocps:
        for mt in range(2):
            oc = ocps.tile([128, S], F32, name=f"oc_{mt}", tag="oc")
            ot = osb.tile([128, S], F32, tag="ot", name=f"ot_{mt}")
            for c in range(2):
                cs = slice(512 * c, 512 * c + 512)
                for n in range(NH):
                    nc.tensor.matmul(
                        oc[:, cs],
                        wo_t[:, n, 128 * mt : 128 * mt + 128],
                        pvu[n][0:CH, cs],
                        start=(n == 0),
                        stop=(n == NH - 1),
                    )
                # b_out is folded into the residual (host adds it to xl)
                nc.vector.tensor_add(
                    ot[:, cs], oc[:, cs], x_t[mt][:, cs].bitcast(F32)
                )
                nc.sync.dma_start(
                    out=out_d[128 * mt : 128 * mt + 128, cs], in_=ot[:, cs]
                )


def _build_nc(loop=False):
    import concourse.bacc as bacc
    import concourse.tile as tile
    from concourse import mybir

    F32 = mybir.dt.float32
    F32R = mybir.dt.float32r
    I32 = mybir.dt.int32

    nc = bacc.Bacc("TRN2", target_bir_lowering=False, debug=False)

    xl_d = nc.dram_tensor("xl", [C, S], F32R, kind="ExternalInput")
    wq_d = nc.dram_tensor("wqT", [C, 256], F32R, kind="ExternalInput")
    wk_d = nc.dram_tensor("wkT", [C, 256], F32R, kind="ExternalInput")
    wv_d = nc.dram_tensor("wvT", [C, 256], F32R, kind="ExternalInput")
    wo_d = nc.dram_tensor("woT", [CH, NH, 256], F32R, kind="ExternalInput")
    br_d = nc.dram_tensor("brows", [1, 4, 256], F32R, kind="ExternalInput")
    on_d = nc.dram_tensor("ones", [128, S], F32R, kind="ExternalInput")
    if loop:
        ni_d = nc.dram_tensor("niter", [1, 1], I32, kind="ExternalInput")
    out_d = nc.dram_tensor("out", [C, S], F32, kind="ExternalOutput")

    with tile.TileContext(nc) as tc:
        with (
            tc.tile_pool(name="wsb", bufs=1) as wsb,
            tc.tile_pool(name="xsb", bufs=1) as xsb,
            tc.tile_pool(name="qksb", bufs=1) as qksb,
            tc.tile_pool(name="vtsb", bufs=8) as vtsb,
            tc.tile_pool(name="sgsb", bufs=2) as sgsb,
            tc.tile_pool(name="etsb", bufs=6) as etsb,
            tc.tile_pool(name="pvsb", bufs=8) as pvsb,
            tc.tile_pool(name="rbsb", bufs=2) as rbsb,
            tc.tile_pool(name="osb", bufs=2) as osb,
        ):
            tc._k_pools = {
                "qksb": qksb,
                "vtsb": vtsb,
                "sgsb": sgsb,
                "etsb": etsb,
                "pvsb": pvsb,
                "rbsb": rbsb,
                "osb": osb,
            }
            # ---- loads -------------------------------------------------
            # every independently-DMA'd piece is its own tile: Tile tracks
            # deps at tile granularity, so consumers must not share a tile
            # with later-arriving data.
            x_t = [
                xsb.tile([128, S], F32R, tag=f"x{i}", name=f"x_t{i}")
                for i in range(2)
            ]
            wq_t = [wsb.tile([128, 256], F32R, tag=f"wq{i}", name=f"wq_t{i}") for i in range(2)]
            wk_t = [wsb.tile([128, 256], F32R, tag=f"wk{i}", name=f"wk_t{i}") for i in range(2)]
            wv_t = [wsb.tile([128, 256], F32R, tag=f"wv{i}", name=f"wv_t{i}") for i in range(2)]
            wo_t = wsb.tile([CH, NH, 256], F32R)
            br_t = wsb.tile([1, 4, 256], F32R)
            on_t = wsb.tile([128, S], F32R)
            # critical-first DMA order: everything the first qk psum group
            # (incl. its closing bias matmul) needs lands first.
            nc.sync.dma_start(out=x_t[0][:, 0:512], in_=xl_d[0:128, 0:512])
            nc.gpsimd.dma_start(out=x_t[1][:, 0:512], in_=xl_d[128:256, 0:512])
            nc.sync.dma_start(out=wq_t[0][:], in_=wq_d[0:128, :])
            nc.gpsimd.dma_start(out=wq_t[1][:], in_=wq_d[128:256, :])
            nc.sync.dma_start(out=br_t[:], in_=br_d[:])
            nc.sync.dma_start(out=on_t[0:33, :], in_=on_d[0:33, :])
            nc.gpsimd.dma_start(out=x_t[1][:, 512:1024], in_=xl_d[128:256, 512:1024])
            nc.sync.dma_start(out=x_t[0][:, 512:1024], in_=xl_d[0:128, 512:1024])
            nc.sync.dma_start(out=wk_t[0][:], in_=wk_d[0:128, :])
            nc.gpsimd.dma_start(out=wk_t[1][:], in_=wk_d[128:256, :])
            nc.sync.dma_start(out=on_t[33:128, :], in_=on_d[33:128, :])
            for kc in range(2):
                nc.gpsimd.dma_start(out=wv_t[kc][:], in_=wv_d[128 * kc : 128 * kc + 128, :])
            nc.gpsimd.dma_start(out=wo_t[:], in_=wo_d[:])

            tiles = (x_t, wq_t, wk_t, wv_t, wo_t, br_t, on_t, out_d)
            if loop:
                ni_t = wsb.tile([1, 1], I32)
                nc.sync.dma_start(out=ni_t[:], in_=ni_d[:])
                niter = nc.values_load(ni_t[0:1, 0:1], min_val=1, max_val=1 << 20)
                with tc.For_i(0, niter, 1):
                    _emit_body(nc, tc, mybir, tiles)
            else:
                _emit_body(nc, tc, mybir, tiles)

    nc.compile()
    return nc


def _get_nc_hw(loop=False):
    key = f"nc_loop{loop}"
    if key not in _CACHE:
        from concourse.bass_interp import get_hw_module

        nc = _build_nc(loop=loop)
        nc.m = get_hw_module(nc.m)
        _CACHE[key] = nc
    return _CACHE[key]


def make_in_maps(x, w_qkv, b_qkv, w_out, b_out):
    """Host-side sharding + weight layout prep. Returns per-core input dicts."""
    f = np.float32
    x = np.ascontiguousarray(np.asarray(x, dtype=f))
    w_qkv = np.asarray(w_qkv, dtype=f)
    b_qkv = np.asarray(b_qkv, dtype=f)
    w_out = np.asarray(w_out, dtype=f)
    b_out = np.asarray(b_out, dtype=f)

    Wr = w_qkv.reshape(NH, 3, CH, C)
    wqT = np.ascontiguousarray(Wr[:, 0].reshape(C, C).T)
    wkT = np.ascontiguousarray(Wr[:, 1].reshape(C, C).T)
    wvT = np.ascontiguousarray(Wr[:, 2].reshape(C, C).T)
    woT = np.ascontiguousarray(w_out.T.reshape(NH, CH, C).transpose(1, 0, 2))
    Br = b_qkv.reshape(NH, 3, CH)
    brows = np.ascontiguousarray(
        np.stack(
            [Br[:, 0].reshape(C), Br[:, 1].reshape(C), Br[:, 2].reshape(C), b_out]
        )[None]
    )
    shared = {
        "wqT": wqT,
        "wkT": wkT,
        "wvT": wvT,
        "woT": woT,
        "brows": brows,
        "ones": np.ones((128, S), dtype=f),
    }
    return [
        {
            "xl": np.ascontiguousarray(x[b].reshape(C, S) + b_out[:, None]),
            **shared,
        }
        for b in range(B)
    ]


def kernel(x, w_qkv, b_qkv, w_out, b_out):
    from concourse.bass_utils import run_bass_kernel_spmd

    nc = _get_nc_hw()
    in_maps = make_in_maps(x, w_qkv, b_qkv, w_out, b_out)
    res = run_bass_kernel_spmd(nc, in_maps, core_ids=list(range(B)), trace=False)
    out = np.stack([res.results[b]["out"].reshape(C, H, W) for b in range(B)])
    return out.astype(np.float32)


if __name__ == "__main__":
    # quick CoreSim logic check on core 0 (no hardware needed)
    from concourse.bass_interp import CoreSim

    sys.path.insert(0, "/root/problem")
    import reference as ref

    inputs = {k: np.asarray(v) for k, v in ref.setup_inputs().items()}
    expected = np.asarray(ref.reference(**inputs))
    in_maps = make_in_maps(**inputs)
    loop = "--loop" in sys.argv
    nc = _build_nc(loop=loop)
    sim = CoreSim(nc)
    for name, arr in in_maps[0].items():
        sim.tensor(name)[:] = arr
    if loop:
        sim.tensor("niter")[:] = 2
    sim.simulate()
    got = np.asarray(sim.tensor("out")).reshape(C, H, W)
    exp0 = expected[0]
    err = np.abs(got - exp0).max() / np.abs(exp0).max()
    print(f"SIM core0 relerr: {err:.3e}")



# revision 30
# speedup vs baseline: 1.3802x; 1.3802x over previous
"""Trainium2 Bass kernel for nn_Attention_7653631722097.

Reference (per batch b of 8, x_b = x[b] as [256, S=1024]):
    qkv = silu(w_qkv @ x_b + b_qkv)                  # [768, S]
    per head n: q,k,v = qkv rows; scores = (k^T q)/sqrt(32)
    attn = softmax(scores, axis=keys); out = attn-weighted v
    out_b = w_out @ hid + b_out + x_b

Distribution: data-parallel, 1 batch per NeuronCore, 8 cores, no collectives.

Engine strategy (per core):
  PE      : all matmuls in fp8e4 DoubleRow where possible
            - qkv projection: host-packed channel pairs (c, c+128)
            - scores: pair-dim stride-0 trick (computes 2*k^T q; folded
              into the exp scale), heads at 32-aligned partition bases
            - PV: transposed layout out[s_block, 33] with a fused ones
              column in the v operand producing the softmax denominator
            - out_proj: bf16 on the DMA-transposed hidden
  Act     : sigmoid for silu, exact Exp (fp8 out) for a share of blocks
  DVE     : silu muls, Schraudolph exp (one tensor_scalar -> int8 bits
            == fp8e4), softmax normalize (reciprocal + broadcast STT),
            residual add
  GPSIMD  : Schraudolph exp share (reads PSUM), small memsets
  DMA     : the [s, nc] -> [nc, s] un-transpose via dma_start_transpose

Accuracy: fp8/bf16/Schraudolph errors land ~1e-4 relative on the output
(residual-dominated); tolerance gate is 2e-2.
"""
import sys

sys.path.insert(0, "/opt/trn_rl_repo")

import numpy as np

B, C, H, W = 8, 256, 32, 32
NH, CH = 8, 32
S = 1024
SCALE = 1.0 / np.sqrt(np.float32(CH))
# scores psum holds 2*k^T q (DoubleRow stride-0 pair trick)
ESC = float(SCALE * 0.5)
# Schraudolph exp -> int8 bits interpreted as fp8e4m3 (bias 7, 3 mantissa)
A8 = float(8.0 / np.log(2.0))
B8 = float(7.0 * 8.0 - 0.3)

# exp block engine split over NBLK blocks
# balance: Act ~1.04us/blk (+7us silu), DVE ~1.19 (+12us other), GP ~1.52
CFG = {
    "EXP_HALF": False,    # half [128,512] exp blocks vs full [128,1024]
    "ST_BUFS": 3,         # psum tiles in flight (half: x1 bank, full: x2)
    "PV_BUFS": 2,
    "QUOTA": (35, 29, 0),
    "DRAIN_GP": False,    # normalize STT on gpsimd instead of DVE
    "PV_MID": False,      # emit PV/drain between score halves
    "ET_BUFS": 4,
    "OUT_BF16": False,
}

_CACHE = {}


def _exp_schedule(nblk):
    """Interleaved engine assignment for the exp blocks. GPSIMD cannot
    read PSUM on real hardware, so only Act/DVE are exp workers."""
    na, nd, ng = CFG["QUOTA"]
    tot = na + nd + ng
    quota = {
        e: n * nblk / tot for e, n in (("a", na), ("d", nd), ("g", ng)) if n > 0
    }
    out = []
    acc = {e: 0.0 for e in quota}
    for _ in range(nblk):
        for e in acc:
            acc[e] += quota[e] / nblk
        pick = max(acc, key=lambda e: acc[e])
        acc[pick] -= 1.0
        out.append(pick)
    return out


def _build_nc(loop=False):
    import concourse.bacc as bacc
    import concourse.tile as tile
    from concourse import mybir

    F32 = mybir.dt.float32
    F32R = mybir.dt.float32r
    BF16 = mybir.dt.bfloat16
    FP8 = mybir.dt.float8e4
    I8 = mybir.dt.int8
    I32 = mybir.dt.int32
    AF = mybir.ActivationFunctionType
    AL = mybir.AluOpType
    DR = mybir.MatmulPerfMode.DoubleRow

    nc = bacc.Bacc("TRN2", target_bir_lowering=False, debug=False)

    x8_d = nc.dram_tensor("x8", [128, 2, S], FP8, kind="ExternalInput")
    xf_d = nc.dram_tensor("xf", [C, S], F32, kind="ExternalInput")
    wq_d = nc.dram_tensor("wq8", [128, 2, 256], FP8, kind="ExternalInput")
    wk_d = nc.dram_tensor("wk8", [128, 2, 256], FP8, kind="ExternalInput")
    wv_d = nc.dram_tensor("wv8", [128, 2, 256], FP8, kind="ExternalInput")
    wo_d = nc.dram_tensor("wo16", [64, 2, 4, 256], BF16, kind="ExternalInput")
    bq_d = nc.dram_tensor("bqkv", [128, 2, 3], F32, kind="ExternalInput")
    bo_d = nc.dram_tensor("bo", [128, 2], F32, kind="ExternalInput")
    bv_d = nc.dram_tensor("bvrow", [1, 512], F32R, kind="ExternalInput")
    on_d = nc.dram_tensor("ones1", [1, 128], F32R, kind="ExternalInput")
    id_d = nc.dram_tensor("id16", [128, 128], F32, kind="ExternalInput")
    if loop:
        ni_d = nc.dram_tensor("niter", [1, 1], I32, kind="ExternalInput")
    out_dt = BF16 if CFG["OUT_BF16"] else F32
    out_d = nc.dram_tensor("out", [C, S], out_dt, kind="ExternalOutput")

    sched = _exp_schedule(128 if CFG["EXP_HALF"] else 64)

    with tile.TileContext(nc) as tc:
        with (
            tc.tile_pool(name="wsb", bufs=1) as wsb,
            tc.tile_pool(name="qk8", bufs=1) as qk8,
            tc.tile_pool(name="v8p", bufs=1) as v8p,
            tc.tile_pool(name="sgp", bufs=2) as sgp,
            tc.tile_pool(name="etp", bufs=CFG["ET_BUFS"]) as etp,
            tc.tile_pool(name="pvn", bufs=1) as pvnp,
            tc.tile_pool(name="osb", bufs=2) as osb,
        ):
            # ---------------- loads ---------------------------------
            x8_t = [
                wsb.tile([128, 2, 512], FP8, tag=f"x8{c}", name=f"x8{c}")
                for c in range(2)
            ]
            xf_t = [wsb.tile([128, S], F32, tag=f"xf{g}", name=f"xf{g}") for g in range(2)]
            wq_t = wsb.tile([128, 2, 256], FP8)
            wk_t = wsb.tile([128, 2, 256], FP8)
            wv_t = wsb.tile([128, 2, 256], FP8)
            wo_t = wsb.tile([64, 2, 4, 256], BF16)
            bq_t = wsb.tile([128, 2, 3], F32)
            bo_t = wsb.tile([128, 2], F32)
            bv_t = wsb.tile([1, 512], F32R)
            on_t = wsb.tile([1, 128], F32R)
            id_t = wsb.tile([128, 128], F32)

            # all DMAs go through the two hardware DGE queues (SP / Act);
            # gpsimd DMA is software-DGE and would burn Pool engine time
            # all DMAs go through the two hardware DGE queues (SP / Act);
            # gpsimd DMA is software-DGE and would burn Pool engine time.
            # per-queue transfers are serial: order by first use.
            nc.sync.dma_start(out=wq_t[:], in_=wq_d[:])
            nc.sync.dma_start(out=x8_t[0][:], in_=x8_d[:, :, 0:512])
            nc.scalar.dma_start(out=x8_t[1][:], in_=x8_d[:, :, 512:S])
            nc.sync.dma_start(out=bq_t[:], in_=bq_d[:])
            nc.sync.dma_start(out=wk_t[:], in_=wk_d[:])
            nc.sync.dma_start(out=wv_t[:], in_=wv_d[:])
            nc.sync.dma_start(out=bv_t[:], in_=bv_d[:])
            nc.sync.dma_start(out=on_t[:], in_=on_d[:])
            nc.scalar.dma_start(out=id_t[:], in_=id_d[:])
            nc.scalar.dma_start(out=wo_t[:], in_=wo_d[:])
            nc.scalar.dma_start(out=bo_t[:], in_=bo_d[:])
            nc.scalar.dma_start(out=xf_t[0][:], in_=xf_d[0:128, :])
            nc.scalar.dma_start(out=xf_t[1][:], in_=xf_d[128:256, :])

            tiles = dict(
                x8=x8_t, xf=xf_t, wq=wq_t, wk=wk_t, wv=wv_t, wo=wo_t,
                bq=bq_t, bo=bo_t, bv=bv_t, on=on_t, id16=id_t, out_d=out_d,
                pools=dict(qk8=qk8, v8p=v8p, sgp=sgp, etp=etp, pvn=pvnp, osb=osb),
                consts=(mybir, F32, BF16, FP8, I8, AF, AL, DR),
                sched=sched,
            )
            if loop:
                ni_t = wsb.tile([1, 1], I32)
                nc.sync.dma_start(out=ni_t[:], in_=ni_d[:])
                niter = nc.values_load(ni_t[0:1, 0:1], min_val=1, max_val=1 << 20)
                with tc.For_i(0, niter, 1):
                    _emit_body(nc, tc, tiles)
            else:
                _emit_body(nc, tc, tiles)

    nc.compile()
    return nc


def _emit_body(nc, tc, t):
    (mybir, F32, BF16, FP8, I8, AF, AL, DR) = t["consts"]
    x8_t, xf_t = t["x8"], t["xf"]
    wq_t, wk_t, wv_t, wo_t = t["wq"], t["wk"], t["wv"], t["wo"]
    bq_t, bo_t, out_d = t["bq"], t["bo"], t["out_d"]
    bv_t, on_t, id_t = t["bv"], t["on"], t["id16"]
    qk8, v8p, sgp, etp, pvnp, osb = (
        t["pools"][k] for k in ("qk8", "v8p", "sgp", "etp", "pvn", "osb")
    )
    sched = t["sched"]

    # persistent sbuf tiles (tagged so the loop variant reuses them)
    q8 = [qk8.tile([128, S], FP8, tag=f"q8{g}", name=f"q8{g}") for g in range(2)]
    k8 = [qk8.tile([128, S], FP8, tag=f"k8{g}", name=f"k8{g}") for g in range(2)]
    v8 = [
        v8p.tile([128, 2, NH, CH + 1], FP8, tag=f"v8{jj}", name=f"v8{jj}")
        for jj in range(4)
    ]
    pvn_t = [
        pvnp.tile([128, 8, 2, CH], F32, tag=f"pvn{i}", name=f"pvn{i}")
        for i in range(4)
    ]
    hid_t = [
        pvnp.tile([64, 2, 4, 128], BF16, tag=f"hid{i}", name=f"hid{i}")
        for i in range(4)
    ]

    # ---------------- phase P: projections + silu -------------------
    with (
        tc.tile_pool(name="qkps", bufs=3, space="PSUM") as qkps,
        tc.tile_pool(name="vps", bufs=2, space="PSUM") as vps,
    ):
        # q/k: psum group g holds channels g*128 + p  (= heads 4g..4g+3)
        for (w_t, dst, pname) in ((wq_t, q8, "q"), (wk_t, k8, "k")):
            bcol = 0 if pname == "q" else 1
            for g in range(2):
                ps = qkps.tile([128, S], F32, name=f"{pname}ps{g}", tag="qkp")
                for c in range(2):
                    cs = slice(512 * c, 512 * c + 512)
                    nc.tensor.matmul(
                        ps[:, cs],
                        w_t[:, :, 128 * g : 128 * g + 128],
                        x8_t[c][:],
                        start=True,
                        stop=True,
                        perf_mode=DR,
                    )
                sg = sgp.tile([128, S], F32, tag="sg", name=f"sg_{pname}{g}")
                nc.scalar.activation(
                    out=sg[:], in_=ps[:], func=AF.Sigmoid,
                    bias=bq_t[:, g, bcol : bcol + 1],
                )
                # silu(z) = z * sigmoid(z); z = psum + bias via STT
                nc.vector.scalar_tensor_tensor(
                    out=dst[g][:],
                    in0=ps[:],
                    scalar=bq_t[:, g, bcol : bcol + 1],
                    in1=sg[:],
                    op0=AL.add,
                    op1=AL.mult,
                )
        # v (transposed): vps[t, o] for two t-blocks per psum tile;
        # the v bias varies along the free (o) dim here, so it enters the
        # psum via a K=1 ones matmul.
        for jj in range(4):
            ps = vps.tile([128, 512], F32, name=f"vps{jj}", tag="vp")
            for e in range(2):
                j = 2 * jj + e
                tsl = slice(128 * (j % 4), 128 * (j % 4) + 128)
                nc.tensor.matmul(
                    ps[:, 256 * e : 256 * e + 256],
                    x8_t[j // 4][:, :, tsl],
                    wv_t[:],
                    start=True,
                    stop=False,
                    perf_mode=DR,
                )
                nc.tensor.matmul(
                    ps[:, 256 * e : 256 * e + 256],
                    on_t[0:1, :],
                    bv_t[0:1, 256 * e : 256 * e + 256],
                    start=False,
                    stop=True,
                )
            sgv = sgp.tile([128, 512], F32, tag="sgv", name=f"sgv{jj}")
            nc.scalar.activation(out=sgv[:], in_=ps[:], func=AF.Sigmoid)
            nc.vector.tensor_mul(
                v8[jj][:, :, :, 0:CH],
                sgv.rearrange("p (e n c) -> p e n c", e=2, n=NH),
                ps.rearrange("p (e n c) -> p e n c", e=2, n=NH),
            )
            nc.gpsimd.memset(v8[jj][:, :, :, CH : CH + 1], 1.0)

    # ---------------- phase A: attention ----------------------------
    with (
        tc.tile_pool(name="stps", bufs=CFG["ST_BUFS"], space="PSUM") as stps,
        tc.tile_pool(name="pvps", bufs=CFG["PV_BUFS"], space="PSUM") as pvps,
    ):
        et_h = {}
        pv_h = {}
        drained = []

        def emit_scores_exp(h, mid=None):
            g, m = divmod(h, 4)
            rs = slice(32 * m, 32 * m + 32)
            ets = [
                etp.tile([128, 2, S], FP8, tag=f"et{(h % 2) * 4 + jj}",
                         name=f"et_{h}_{jj}")
                for jj in range(4)
            ]
            et_h[h] = ets
            def emit_exp(eng, eout, st_ap, _alt=[0]):
                if eng == "a" and h == 0:
                    # keep Act on the sigmoid table until phase P fully
                    # drains; head-0 blocks go to the DVE Schraudolph path
                    eng = "d"
                if eng == "a":
                    nc.scalar.activation(
                        out=eout, in_=st_ap, func=AF.Exp, scale=ESC
                    )
                elif eng == "d":
                    nc.vector.tensor_scalar(
                        out=eout.bitcast(I8),
                        in0=st_ap,
                        scalar1=float(A8 * ESC),
                        scalar2=float(B8),
                        op0=AL.mult,
                        op1=AL.add,
                    )
                else:
                    nc.gpsimd.tensor_scalar(
                        out=eout.bitcast(I8),
                        in0=st_ap,
                        scalar1=float(A8 * ESC),
                        scalar2=float(B8),
                        op0=AL.mult,
                        op1=AL.add,
                    )

            for j in range(8):
                tsl = slice(128 * j, 128 * j + 128)
                if CFG["PV_MID"] and j == 4 and mid is not None:
                    mid()
                if CFG["EXP_HALF"]:
                    for c in range(2):
                        eng = sched[h * 16 + 2 * j + c]
                        cs = slice(512 * c, 512 * c + 512)
                        st = stps.tile(
                            [128, 512], F32, name=f"st_{h}_{j}_{c}", tag="st"
                        )
                        nc.tensor.matmul(
                            st[:],
                            k8[g][rs, None, tsl].to_broadcast((32, 2, 128)),
                            q8[g][rs, None, cs].to_broadcast((32, 2, 512)),
                            start=True,
                            stop=True,
                            perf_mode=DR,
                            tile_position=(32 * m, 0),
                        )
                        emit_exp(eng, ets[j // 2][:, j % 2, cs], st[:])
                else:
                    eng = sched[h * 8 + j]
                    st = stps.tile(
                        [128, S], F32, name=f"st_{h}_{j}", tag="st"
                    )
                    for c in range(2):
                        cs = slice(512 * c, 512 * c + 512)
                        nc.tensor.matmul(
                            st[:, cs],
                            k8[g][rs, None, tsl].to_broadcast((32, 2, 128)),
                            q8[g][rs, None, cs].to_broadcast((32, 2, 512)),
                            start=True,
                            stop=True,
                            perf_mode=DR,
                            tile_position=(32 * m, 0),
                        )
                    emit_exp(eng, ets[j // 2][:, j % 2, :], st[:])

        def emit_pv(h):
            pv = pvps.tile([128, 8, CH + 1], F32, name=f"pv_{h}", tag="pv")
            pv_h[h] = pv
            ets = et_h[h]
            for sb in range(8):
                ssl = slice(128 * sb, 128 * sb + 128)
                for jj in range(4):
                    nc.tensor.matmul(
                        pv[:, sb, :],
                        ets[jj][:, :, ssl],
                        v8[jj][:, :, h, :],
                        start=(jj == 0),
                        stop=(jj == 3),
                        perf_mode=DR,
                    )

        def emit_drain(h):
            pv = pv_h.pop(h)
            rs = pvnp.tile([128, 8], F32, tag=f"rs{h % 2}", name=f"rs_{h}")
            with nc.allow_low_precision(reason="softmax denom reciprocal"):
                nc.vector.reciprocal(out=rs[:], in_=pv[:, :, CH])
            deng = nc.gpsimd if CFG["DRAIN_GP"] else nc.vector
            deng.scalar_tensor_tensor(
                out=pvn_t[h // 2][:, :, h % 2, :],
                in0=pv[:, :, 0:CH],
                scalar=1.0,
                in1=rs[:, :, None].to_broadcast((128, 8, CH)),
                op0=AL.mult,
                op1=AL.mult,
            )
            del et_h[h]

        def mk_mid(hh):
            def mid():
                emit_pv(hh)
                emit_drain(hh)
            return mid

        for h in range(NH):
            if CFG["PV_MID"]:
                emit_scores_exp(h, mid=mk_mid(h - 1) if h > 0 else None)
            else:
                emit_scores_exp(h)
                if h > 0:
                    emit_pv(h - 1)
                    emit_drain(h - 1)
        emit_pv(NH - 1)
        emit_drain(NH - 1)

    # ---------------- phase O: un-transpose + projection + residual --
    # PE transpose per (quarter, s-block): pvn[q][:, sb, :, :] [128 s, 64]
    # -> psum [64, 128] at partition base 64*(sb%2); then one Act copy per
    # quarter into hid_t[q] (the DmaTransposeAnt path hangs the runtime).
    out_dt = BF16 if CFG["OUT_BF16"] else F32
    ot_g = [osb.tile([128, S], out_dt, tag=f"otg{g}", name=f"otg{g}") for g in range(2)]
    with (
        tc.tile_pool(name="htp", bufs=4, space="PSUM") as htp,
        tc.tile_pool(name="ocps", bufs=2, space="PSUM") as ocps,
    ):
        for q in range(4):
            for par in range(2):
                hp = htp.tile([64, 4, 128], F32, name=f"hp{q}{par}", tag="hp")
                for ee in range(4):
                    sb = 2 * ee + par
                    nc.tensor.transpose(
                        out=hp[:, ee, :],
                        in_=pvn_t[q][:, sb, :, :],
                        identity=id_t[:],
                    )
                nc.scalar.copy(hid_t[q][:, par, :, :], hp[:])
        for g in range(2):
            for cs in range(2):
                ssl = slice(512 * cs, 512 * cs + 512)
                oc = ocps.tile([128, 512], F32, name=f"oc{g}{cs}", tag="oc")
                for sbi in range(4):
                    sb = 4 * cs + sbi
                    ee, par = sb // 2, sb % 2
                    for q in range(4):
                        nc.tensor.matmul(
                            oc[:, 128 * sbi : 128 * sbi + 128],
                            wo_t[:, par, q, 128 * g : 128 * g + 128],
                            hid_t[q][:, par, ee, :],
                            start=(q == 0),
                            stop=(q == 3),
                        )
                nc.vector.scalar_tensor_tensor(
                    out=ot_g[g][:, ssl],
                    in0=oc[:],
                    scalar=bo_t[:, g : g + 1],
                    in1=xf_t[g][:, ssl],
                    op0=AL.add,
                    op1=AL.add,
                )
        for g in range(2):
            qeng = (nc.sync, nc.scalar)[g]
            qeng.dma_start(out=out_d[128 * g : 128 * g + 128, :], in_=ot_g[g][:])


def _get_nc_hw(loop=False):
    key = f"nc_loop{loop}"
    if key not in _CACHE:
        from concourse.bass_interp import get_hw_module

        nc = _build_nc(loop=loop)
        nc.m = get_hw_module(nc.m)
        _CACHE[key] = nc
    return _CACHE[key]


def make_in_maps(x, w_qkv, b_qkv, w_out, b_out):
    """Host-side sharding + weight packing. Returns per-core input dicts."""
    from concourse import mybir

    f = np.float32
    f8 = mybir.dt.np(mybir.dt.float8e4)
    bf = mybir.dt.np(mybir.dt.bfloat16)

    x = np.ascontiguousarray(np.asarray(x, dtype=f))        # [B, C, H, W]
    w_qkv = np.asarray(w_qkv, dtype=f)                      # [768, 256]
    b_qkv = np.asarray(b_qkv, dtype=f)                      # [768]
    w_out = np.asarray(w_out, dtype=f)                      # [256, 256]
    b_out = np.asarray(b_out, dtype=f)                      # [256]

    Wr = w_qkv.reshape(NH, 3, CH, C)
    wq = Wr[:, 0].reshape(C, C)   # [o=h*32+c, cin]
    wk = Wr[:, 1].reshape(C, C)
    wv = Wr[:, 2].reshape(C, C)
    Br = b_qkv.reshape(NH, 3, CH)

    def pack_pairs_lhs(wm):
        # lhsT [p, e, ocol]: contraction row cin = e*128 + p
        wt = wm.T  # [cin, o]
        return np.ascontiguousarray(
            wt.reshape(2, 128, C).transpose(1, 0, 2).astype(f8)
        )

    wq8 = pack_pairs_lhs(wq)
    wk8 = pack_pairs_lhs(wk)
    wv8 = pack_pairs_lhs(wv)
    woq = w_out.T.reshape(4, 64, C)          # [q, r64, o]
    wo16 = np.ascontiguousarray(
        np.stack([woq, woq], axis=0)          # [par(dup), q, r64, o]
        .transpose(2, 0, 1, 3)                # [r64, par, q, o]
        .astype(bf)
    )
    bqkv = np.ascontiguousarray(
        np.stack(
            [Br[:, t].reshape(C).reshape(2, 128).T for t in range(3)], axis=2
        )
    ).astype(f)  # [128, 2(g), 3(qkv)]
    bo = np.ascontiguousarray(b_out.reshape(2, 128).T).astype(f)  # [128, 2]

    bvrow = np.ascontiguousarray(
        np.tile(Br[:, 2].reshape(C), 2)[None, :]
    ).astype(f)  # [1, 512]
    ones1 = np.ones((1, 128), dtype=f)
    id16 = np.eye(128, dtype=f)
    shared = {"wq8": wq8, "wk8": wk8, "wv8": wv8, "wo16": wo16,
              "bqkv": bqkv, "bo": bo, "bvrow": bvrow, "ones1": ones1,
              "id16": id16}
    maps = []
    for b in range(B):
        xb = x[b].reshape(C, S)
        x8 = np.ascontiguousarray(
            xb.reshape(2, 128, S).transpose(1, 0, 2).astype(f8)
        )
        maps.append({"x8": x8, "xf": np.ascontiguousarray(xb), **shared})
    return maps


def kernel(x, w_qkv, b_qkv, w_out, b_out):
    from concourse.bass_utils import run_bass_kernel_spmd

    nc = _get_nc_hw()
    in_maps = make_in_maps(x, w_qkv, b_qkv, w_out, b_out)
    res = run_bass_kernel_spmd(nc, in_maps, core_ids=list(range(B)), trace=False)
    out = np.stack([res.results[b]["out"].reshape(C, H, W) for b in range(B)])
    return out.astype(np.float32)


if __name__ == "__main__":
    # quick CoreSim logic check on core 0 (no hardware needed)
    from concourse.bass_interp import CoreSim

    sys.path.insert(0, "/root/problem")
    import jax
    import reference as ref

    with jax.default_device(jax.devices("cpu")[0]):
        inputs = {k: np.asarray(v) for k, v in ref.setup_inputs().items()}
        expected = np.asarray(ref.reference(**inputs))
    in_maps = make_in_maps(**inputs)
    loop = "--loop" in sys.argv
    nc = _build_nc(loop=loop)
    sim = CoreSim(nc)
    for name, arr in in_maps[0].items():
        sim.tensor(name)[:] = arr
    if loop:
        sim.tensor("niter")[:] = 2
    sim.simulate()
    got = np.asarray(sim.tensor("out")).reshape(C, H, W)
    exp0 = expected[0]
    err = np.abs(got - exp0).max() / np.abs(exp0).max()
    print(f"SIM core0 relerr: {err:.3e}")
